# revision 15
# baseline (speedup 1.0000x reference)
"""CrossScan (4-directional) Trainium2 Bass kernel.

Input  x:   [16, 96, 128, 128] f32
Output out: [16, 4, 96, 16384] f32
  out[b,0,c] = x[b,c] flattened row-major
  out[b,1,c] = x[b,c].T flattened
  out[b,2,c] = reverse(out[b,0,c])
  out[b,3,c] = reverse(out[b,1,c])

Strategy: shard batch across 8 cores (2 samples = 192 (b,c)-planes per core,
no communication). Plane-per-partition layout: each 128x128 plane lives
entirely in one SBUF partition's free axis (64KB), so transpose and reversal
are within-partition strided-AP copies on DVE/ACT.

DMA on TRN2 runs ~2x slower for transfers that don't reach all 16 SBUF AXI
ports (port = bits[4:2]<<1 | bit[6] of the partition index), so:
  - T1 holds planes 0..127 -> every T1 transfer spans 128 partitions.
  - T2 holds the remaining 64 planes at partitions 32..95 (straddling the
    bit-6 boundary -> all 16 ports, ~374 GB/s instead of ~190).
  - T2's four output scans are packed PAIRWISE into [128, CH] staging tiles
    using partition-shifted engine writes (DVE/ACT read lanes 32..95, write
    lanes 0..63 or 64..127), so every store is a full 128-partition DMA.
Output is a kernel-private slot layout [6, 128, HW]; the host remaps slots
back to [2, 4, 96, HW] per core. Total traffic = 60 MiB/core (the minimum).
"""

import sys

for _p in ("/opt/trn_rl_repo",):
    if _p not in sys.path:
        sys.path.insert(0, _p)

import numpy as np

B, C, H, W = 16, 96, 128, 128
HW = H * W
N_CORES = 8
B_PER = B // N_CORES   # 2 samples per core
NPLANES = B_PER * C    # 192 planes per core
NT2 = NPLANES - 128    # 64 planes in the remainder tile

CH = 4096            # free elements per staged chunk (16KB/partition)
NCHUNK = HW // CH    # 4 chunks per output stream
WBLK = CH // H       # 32 w-columns per transpose chunk

_cache = {}


def _views(inv):
    """(transpose, reverse, reverse-transpose) read APs for a [P, HW] tile view."""
    tr = inv.rearrange("p (h w) -> p h w", w=W).transpose([0, 2, 1])  # [P][w][h]
    rev = inv[:, ::-1]
    part = list(inv.ap[0])
    revtr = inv.__replace__(
        offset=inv.offset + HW - 1, ap=[part, [-1, W], [-W, H]]
    )  # [P][w'][h'] reads HW-1 - w' - W*h'
    return tr, rev, revtr


def _build_nc():
    import concourse.bacc as bacc
    import concourse.mybir as mybir
    from concourse.tile import TileContext

    f32 = mybir.dt.float32
    nc = bacc.Bacc("TRN2", target_bir_lowering=False, debug=False)
    x = nc.declare_dram_parameter("x", [B_PER, C, H, W], f32, isOutput=False)
    out = nc.declare_dram_parameter("out", [6, 128, HW], f32, isOutput=True)

    planes = x[:].rearrange("b c h w -> (b c) (h w)")  # [192, HW] DRAM view

    with TileContext(nc) as tc:
        with (
            tc.tile_pool(name="inp", bufs=2) as inp_pool,
            tc.tile_pool(name="stage", bufs=4) as st_pool,
        ):
            # ---- loads ----
            t1 = inp_pool.tile([128, HW], f32, tag="in")
            nc.sync.dma_start(out=t1[:], in_=planes[0:128])
            t2 = inp_pool.tile([128, HW], f32, tag="in")
            nc.sync.dma_start(out=t2[32 : 32 + NT2, :], in_=planes[128:NPLANES])

            inv1 = t1[:]
            tr1, rev1, revtr1 = _views(inv1)
            inv2 = t2[:]
            tr2, rev2, revtr2 = _views(inv2)

            # ---- T1 scan0: direct 8 MiB store ----
            nc.sync.dma_start(out=out[0], in_=inv1)

            # ---- chunked streams ----
            for j in range(NCHUNK):
                jc = slice(j * CH, (j + 1) * CH)
                jw = slice(j * WBLK, (j + 1) * WBLK)

                # T2 pair A: lanes 0..63 = scan0 (copy, DVE), 64..127 = scan3
                # (reverse-transpose, ACT), partition-shifted from lanes 32..95.
                # Engine reads may not cross the 64-partition base boundary, so
                # each T2 stream is split into two 32-lane instructions.
                pa = st_pool.tile([128, CH], f32, tag="st")
                for lo, n in ((0, 32), (32, 32)):
                    src = slice(32 + lo, 32 + lo + n)
                    nc.vector.tensor_copy(pa[lo : lo + n, :], inv2[src, jc])
                    nc.scalar.copy(
                        pa[64 + lo : 64 + lo + n, :], revtr2[src, jw, :]
                    )
                nc.sync.dma_start(out=out[4, :, jc], in_=pa[:])

                # T1 scan2 (reverse) on ACT
                a2 = st_pool.tile([128, CH], f32, tag="st")
                nc.scalar.copy(a2[:], rev1[:, jc])
                nc.sync.dma_start(out=out[2, :, jc], in_=a2[:])

                # T1 scan1 (transpose) on DVE
                a1 = st_pool.tile([128, CH], f32, tag="st")
                nc.vector.tensor_copy(a1[:], tr1[:, jw, :])
                nc.sync.dma_start(out=out[1, :, jc], in_=a1[:])

                # T1 scan3 (reverse-transpose) on DVE
                a3 = st_pool.tile([128, CH], f32, tag="st")
                nc.vector.tensor_copy(a3[:], revtr1[:, jw, :])
                nc.sync.dma_start(out=out[3, :, jc], in_=a3[:])

                # T2 pair B: lanes 0..63 = scan1 (transpose, DVE), 64..127 =
                # scan2 (reverse, DVE), partition-shifted from lanes 32..95.
                pb = st_pool.tile([128, CH], f32, tag="st")
                for lo, n in ((0, 32), (32, 32)):
                    src = slice(32 + lo, 32 + lo + n)
                    nc.vector.tensor_copy(pb[lo : lo + n, :], tr2[src, jw, :])
                    nc.vector.tensor_copy(pb[64 + lo : 64 + lo + n, :], rev2[src, jc])
                nc.sync.dma_start(out=out[5, :, jc], in_=pb[:])
    nc.compile()
    return nc


def _get_nc():
    if "nc" not in _cache:
        _cache["nc"] = _build_nc()
    return _cache["nc"]


def _unscramble(lin):
    """[6, 128, HW] core output -> [B_PER, 4, C, HW]."""
    o = np.empty((B_PER, 4, C, HW), dtype=lin.dtype)
    for s in range(4):
        o[0, s] = lin[s, :C]          # planes 0..95  = b0 c0..95
        o[1, s, :32] = lin[s, C:128]  # planes 96..127 = b1 c0..31
    # planes 128..191 = b1 c32..95
    o[1, 0, 32:] = lin[4, :NT2]
    o[1, 3, 32:] = lin[4, 64 : 64 + NT2]
    o[1, 1, 32:] = lin[5, :NT2]
    o[1, 2, 32:] = lin[5, 64 : 64 + NT2]
    return o


def _run(x_np, trace=False):
    import time

    from concourse.bass_utils import run_bass_kernel_spmd

    nc = _get_nc()
    x_np = np.ascontiguousarray(x_np, dtype=np.float32)
    in_maps = [
        {"x": np.ascontiguousarray(x_np[i * B_PER : (i + 1) * B_PER])}
        for i in range(N_CORES)
    ]
    last_err = None
    for attempt in range(3):
        try:
            res = run_bass_kernel_spmd(nc, in_maps, list(range(N_CORES)), trace=trace)
            break
        except Exception as e:  # transient NRT_EXEC_UNIT_UNRECOVERABLE seen rarely
            last_err = e
            time.sleep(15)
    else:
        raise last_err
    full = np.concatenate([_unscramble(r["out"]) for r in res.results], axis=0)
    return full, res


def kernel(x):
    full, _ = _run(x, trace=False)
    return full


def kernel_profiled(x):
    """Returns (output, exec_time_ns, BassKernelResults) — used by test.py only."""
    full, res = _run(x, trace=True)
    return full, res.exec_time_ns, res


# revision 16
# speedup vs baseline: 1.0979x; 1.0979x over previous
"""CrossScan (4-directional) Trainium2 Bass kernel.

Input  x:   [16, 96, 128, 128] f32
Output out: [16, 4, 96, 16384] f32
  out[b,0,c] = x[b,c] flattened row-major
  out[b,1,c] = x[b,c].T flattened
  out[b,2,c] = reverse(out[b,0,c])
  out[b,3,c] = reverse(out[b,1,c])

Strategy: shard batch across 8 cores (2 samples = 192 (b,c)-planes per core,
no communication). Plane-per-partition layout: each 128x128 plane lives
entirely in one SBUF partition's free axis (64KB), so transpose and reversal
are within-partition strided-AP copies on DVE/ACT.

DMA on TRN2 runs ~2x slower for transfers that don't reach all 16 SBUF AXI
ports (port = bits[4:2]<<1 | bit[6] of the partition index), so:
  - T1 holds planes 0..127 -> every T1 transfer spans 128 partitions.
  - T2 holds the remaining 64 planes at partitions 32..95 (straddling the
    bit-6 boundary -> all 16 ports, ~374 GB/s instead of ~190).
  - T2's four output scans are packed PAIRWISE into [128, CH] staging tiles
    using partition-shifted engine writes (DVE/ACT read lanes 32..95, write
    lanes 0..63 or 64..127), so every store is a full 128-partition DMA.
Output is a kernel-private slot layout [6, 128, HW]; the host remaps slots
back to [2, 4, 96, HW] per core. Total traffic = 60 MiB/core (the minimum).
"""

import sys

for _p in ("/opt/trn_rl_repo",):
    if _p not in sys.path:
        sys.path.insert(0, _p)

import numpy as np

B, C, H, W = 16, 96, 128, 128
HW = H * W
N_CORES = 8
B_PER = B // N_CORES   # 2 samples per core
NPLANES = B_PER * C    # 192 planes per core
NT2 = NPLANES - 128    # 64 planes in the remainder tile

CH = 4096            # free elements per staged chunk (16KB/partition)
NCHUNK = HW // CH    # 4 chunks per output stream
WBLK = CH // H       # 32 w-columns per transpose chunk

_cache = {}


def _views(inv):
    """(transpose, reverse, reverse-transpose) read APs for a [P, HW] tile view."""
    tr = inv.rearrange("p (h w) -> p h w", w=W).transpose([0, 2, 1])  # [P][w][h]
    rev = inv[:, ::-1]
    part = list(inv.ap[0])
    revtr = inv.__replace__(
        offset=inv.offset + HW - 1, ap=[part, [-1, W], [-W, H]]
    )  # [P][w'][h'] reads HW-1 - w' - W*h'
    return tr, rev, revtr


def _build_nc():
    import concourse.bacc as bacc
    import concourse.mybir as mybir
    from concourse.tile import TileContext

    f32 = mybir.dt.float32
    nc = bacc.Bacc("TRN2", target_bir_lowering=False, debug=False)
    x = nc.declare_dram_parameter("x", [B_PER, C, H, W], f32, isOutput=False)
    out = nc.declare_dram_parameter("out", [6, 128, HW], f32, isOutput=True)

    planes = x[:].rearrange("b c h w -> (b c) (h w)")  # [192, HW] DRAM view

    with TileContext(nc) as tc:
        with (
            tc.tile_pool(name="inp", bufs=2) as inp_pool,
            tc.tile_pool(name="stage", bufs=4) as st_pool,
        ):
            # ---- loads ----
            t2 = inp_pool.tile([128, HW], f32, tag="in")
            nc.sync.dma_start(out=t2[32 : 32 + NT2, :], in_=planes[128:NPLANES])
            t1 = inp_pool.tile([128, HW], f32, tag="in")
            nc.sync.dma_start(out=t1[:], in_=planes[0:128])

            inv1 = t1[:]
            tr1, rev1, revtr1 = _views(inv1)
            inv2 = t2[:]
            tr2, rev2, revtr2 = _views(inv2)

            # ---- T1 scan0: direct 8 MiB store ----
            nc.sync.dma_start(out=out[0], in_=inv1)

            # ---- chunked streams ----
            for j in range(NCHUNK):
                jc = slice(j * CH, (j + 1) * CH)
                jw = slice(j * WBLK, (j + 1) * WBLK)

                # T2 pair A: lanes 0..63 = scan0 (copy, DVE), 64..127 = scan3
                # (reverse-transpose, ACT), partition-shifted from lanes 32..95.
                # Engine reads may not cross the 64-partition base boundary, so
                # each T2 stream is split into two 32-lane instructions.
                pa = st_pool.tile([128, CH], f32, tag="st")
                nc.vector.tensor_copy(pa[0:32, :], inv2[32:64, jc])
                nc.scalar.copy(pa[32:64, :], inv2[64:96, jc])
                nc.scalar.copy(pa[64:96, :], revtr2[32:64, jw, :])
                nc.scalar.copy(pa[96:128, :], revtr2[64:96, jw, :])
                nc.sync.dma_start(out=out[4, :, jc], in_=pa[:])

                # T1 scan2 (reverse) on ACT
                a2 = st_pool.tile([128, CH], f32, tag="st")
                nc.scalar.copy(a2[:], rev1[:, jc])
                nc.sync.dma_start(out=out[2, :, jc], in_=a2[:])

                # T1 scan1 (transpose) on DVE
                a1 = st_pool.tile([128, CH], f32, tag="st")
                nc.vector.tensor_copy(a1[:], tr1[:, jw, :])
                nc.sync.dma_start(out=out[1, :, jc], in_=a1[:])

                # T1 scan3 (reverse-transpose) on DVE
                a3 = st_pool.tile([128, CH], f32, tag="st")
                nc.vector.tensor_copy(a3[:], revtr1[:, jw, :])
                nc.sync.dma_start(out=out[3, :, jc], in_=a3[:])

                # T2 pair B: lanes 0..63 = scan1 (transpose, DVE), 64..127 =
                # scan2 (reverse, DVE), partition-shifted from lanes 32..95.
                pb = st_pool.tile([128, CH], f32, tag="st")
                for lo, n in ((0, 32), (32, 32)):
                    src = slice(32 + lo, 32 + lo + n)
                    nc.vector.tensor_copy(pb[lo : lo + n, :], tr2[src, jw, :])
                    nc.vector.tensor_copy(pb[64 + lo : 64 + lo + n, :], rev2[src, jc])
                nc.sync.dma_start(out=out[5, :, jc], in_=pb[:])
    nc.compile()
    return nc


def _get_nc():
    if "nc" not in _cache:
        _cache["nc"] = _build_nc()
    return _cache["nc"]


def _unscramble(lin):
    """[6, 128, HW] core output -> [B_PER, 4, C, HW]."""
    o = np.empty((B_PER, 4, C, HW), dtype=lin.dtype)
    for s in range(4):
        o[0, s] = lin[s, :C]          # planes 0..95  = b0 c0..95
        o[1, s, :32] = lin[s, C:128]  # planes 96..127 = b1 c0..31
    # planes 128..191 = b1 c32..95
    o[1, 0, 32:] = lin[4, :NT2]
    o[1, 3, 32:] = lin[4, 64 : 64 + NT2]
    o[1, 1, 32:] = lin[5, :NT2]
    o[1, 2, 32:] = lin[5, 64 : 64 + NT2]
    return o


def _run(x_np, trace=False):
    import time

    from concourse.bass_utils import run_bass_kernel_spmd

    nc = _get_nc()
    x_np = np.ascontiguousarray(x_np, dtype=np.float32)
    in_maps = [
        {"x": np.ascontiguousarray(x_np[i * B_PER : (i + 1) * B_PER])}
        for i in range(N_CORES)
    ]
    last_err = None
    for attempt in range(3):
        try:
            res = run_bass_kernel_spmd(nc, in_maps, list(range(N_CORES)), trace=trace)
            break
        except Exception as e:  # transient NRT_EXEC_UNIT_UNRECOVERABLE seen rarely
            last_err = e
            time.sleep(15)
    else:
        raise last_err
    full = np.concatenate([_unscramble(r["out"]) for r in res.results], axis=0)
    return full, res


def kernel(x):
    full, _ = _run(x, trace=False)
    return full


def kernel_profiled(x):
    """Returns (output, exec_time_ns, BassKernelResults) — used by test.py only."""
    full, res = _run(x, trace=True)
    return full, res.exec_time_ns, res
